# revision 1
# baseline (speedup 1.0000x reference)
"""Trainium2 Bass kernel for DifferentialDualAttentionInteractiveBlock.

Self-contained: hardcodes shapes (nW=1024, N=64, DIM=192, H=6, d=32),
shards data-parallel over windows across 8 NeuronCores.
"""
import sys

sys.path.insert(0, "/opt/trn_rl_repo")

import numpy as np
import ml_dtypes

import concourse.bass as bass  # noqa: F401
import concourse.bacc as bacc
import concourse.mybir as mybir
from concourse.tile import TileContext

BF16 = ml_dtypes.bfloat16
F32 = np.float32

WS = 8
N = 64
DIM = 192
H = 6
HD = 32
SCALE = HD ** -0.5
N_CORES = 8
NW = 1024
NWC = NW // N_CORES
GROUP = 8
UNITS_PER_GROUP = GROUP // 2
N_GROUPS = NWC // GROUP
TOK_C = NWC * N

_COMPILED = {}


def _rel_pos_bias(rpb_table):
    coords = np.stack(np.meshgrid(np.arange(WS), np.arange(WS), indexing="ij"))
    cf = coords.reshape(2, -1)
    rc = (cf[:, :, None] - cf[:, None, :]).transpose(1, 2, 0).astype(np.int64)
    rc[:, :, 0] += WS - 1
    rc[:, :, 1] += WS - 1
    rc[:, :, 0] *= 2 * WS - 1
    idx = rc.sum(-1)
    bias = np.asarray(rpb_table)[idx.reshape(-1)].reshape(N, N, H)
    return bias.transpose(2, 0, 1).astype(np.float64)  # [H, q, k]


def _sigmoid_clip(lam):
    s = 1.0 / (1.0 + np.exp(-np.float64(lam)))
    return float(np.clip(s, 0.01, 0.99))


def _strip(h, br):
    """(row-strip, index-within-strip) for head h, branch br."""
    if h < 4:
        return h, 0
    return (h - 4) + 2 * br, 1


def _host_prep(inputs):
    x = np.asarray(inputs["x_windows"], dtype=F32)
    y = np.asarray(inputs["y_windows"], dtype=F32)

    qkv = np.asarray(inputs["sa_qkv_w"], dtype=np.float64)
    sa_ct = np.asarray(inputs["sa_ct_w"], dtype=np.float64)
    sa_cr = np.asarray(inputs["sa_cr_w"], dtype=np.float64)
    ca_q = np.asarray(inputs["ca_q_w"], dtype=np.float64)
    ca_kv = np.asarray(inputs["ca_kv_w"], dtype=np.float64)
    ca_ct = np.asarray(inputs["ca_ct_w"], dtype=np.float64)
    ca_cr = np.asarray(inputs["ca_cr_w"], dtype=np.float64)
    sa_enh = float(np.asarray(inputs["sa_enh"]))
    ca_enh = float(np.asarray(inputs["ca_enh"]))

    Wq_sa = qkv[0:DIM] * SCALE
    Wk_sa = qkv[DIM:2 * DIM]
    Wv_sa = qkv[2 * DIM:3 * DIM]
    Wq_ca = ca_q * SCALE
    Wk_ca = ca_kv[0:DIM]
    Wv_ca = ca_kv[DIM:2 * DIM]

    def ct(W, enh, cross):
        return np.concatenate([W, enh * (W @ cross)], axis=1)

    def cr(W, enh, cross):
        return np.concatenate([enh * (W @ cross), W], axis=1)

    W_fm_t = np.concatenate([
        ct(Wq_sa, sa_enh, sa_cr), ct(Wk_sa, sa_enh, sa_cr),
        ct(Wq_ca, ca_enh, ca_cr), ct(Wk_ca, ca_enh, ca_cr)], axis=0)
    W_fm_r = np.concatenate([
        cr(Wq_sa, sa_enh, sa_ct), cr(Wk_sa, sa_enh, sa_ct),
        cr(Wq_ca, ca_enh, ca_ct), cr(Wk_ca, ca_enh, ca_ct)], axis=0)
    # permute rows so q-head-h and k-head-h share partition offsets:
    # oc0=q_sa h0-3, oc1=k_sa h0-3, oc2=[q4s,q5s,q4c,q5c], oc3=[k4s,k5s,k4c,k5c],
    # oc4=q_ca h0-3, oc5=k_ca h0-3
    perm = np.concatenate([
        np.arange(0, 128), np.arange(192, 320),
        np.arange(128, 192), np.arange(384 + 128, 384 + 192),
        np.arange(320, 384), np.arange(576 + 128, 576 + 192),
        np.arange(384, 512), np.arange(576, 704)])
    W_fm_t = W_fm_t[perm]
    W_fm_r = W_fm_r[perm]
    wfm = np.zeros((2, 6, 3, 128, 128), dtype=BF16)
    for s, W in enumerate([W_fm_t, W_fm_r]):
        for oc in range(6):
            for kc in range(3):
                blk = W[oc * 128:(oc + 1) * 128, kc * 128:(kc + 1) * 128]
                wfm[s, oc, kc] = blk.T.astype(BF16)

    W_v_t = np.concatenate([ct(Wv_sa, sa_enh, sa_cr),
                            ct(Wv_ca, ca_enh, ca_cr)], axis=0)
    W_v_r = np.concatenate([cr(Wv_sa, sa_enh, sa_ct),
                            cr(Wv_ca, ca_enh, ca_ct)], axis=0)
    # wv[kc] : [128, 768] = [t-side (384) | r-side (384)]
    wv = np.zeros((3, 128, 768), dtype=BF16)
    for kc in range(3):
        wv[kc, :, 0:384] = W_v_t[:, kc * 128:(kc + 1) * 128].T.astype(BF16)
        wv[kc, :, 384:768] = W_v_r[:, kc * 128:(kc + 1) * 128].T.astype(BF16)

    bias = _rel_pos_bias(inputs["rpb_table"])
    # exp(bias), both branches: col = strip*512 + br*256 + idx2*128 + s*64
    eb = np.ones((128, 2048), dtype=BF16)
    for br in range(2):
        for h in range(H):
            st, i2 = _strip(h, br)
            e = np.exp(bias[h]).T.astype(BF16)  # [k, q]
            for s in range(2):
                c = st * 512 + br * 256 + i2 * 128 + s * 64
                eb[0:64, c:c + 64] = e
                eb[64:128, c:c + 64] = e

    pt = np.zeros((2, 192, 192), dtype=BF16)
    pt[0] = np.asarray(inputs["proj_sa_w"], dtype=np.float64).T.astype(BF16)
    pt[1] = np.asarray(inputs["proj_ca_w"], dtype=np.float64).T.astype(BF16)

    ident = np.eye(128, dtype=BF16)

    lam_sa = _sigmoid_clip(inputs["lambda_sa"])
    lam_ca = _sigmoid_clip(inputs["lambda_ca"])

    zs = []
    for c in range(N_CORES):
        z = np.concatenate([x[c * NWC:(c + 1) * NWC], y[c * NWC:(c + 1) * NWC]],
                           axis=-1)
        zT = z.reshape(NWC * N, 384).T.astype(BF16).reshape(3, 128, TOK_C)
        zs.append(np.ascontiguousarray(zT))

    shared = {"wfm": wfm, "wv": wv, "eb": eb, "pt": pt, "ident": ident}
    return shared, zs, (lam_sa, lam_ca)


def _build_nc(lam_sa, lam_ca, nwc=NWC):
    n_groups = nwc // GROUP
    tok_c = nwc * N
    nc = bacc.Bacc(None, target_bir_lowering=False)
    bf = mybir.dt.bfloat16
    f32 = mybir.dt.float32
    Exp = mybir.ActivationFunctionType.Exp

    zt_d = nc.declare_dram_parameter("zt", [3, 128, tok_c], bf, isOutput=False)
    wfm_d = nc.declare_dram_parameter("wfm", [2, 6, 3, 128, 128], bf, isOutput=False)
    wv_d = nc.declare_dram_parameter("wv", [3, 128, 768], bf, isOutput=False)
    eb_d = nc.declare_dram_parameter("eb", [128, 2048], bf, isOutput=False)
    pt_d = nc.declare_dram_parameter("pt", [2, 192, 192], bf, isOutput=False)
    id_d = nc.declare_dram_parameter("ident", [128, 128], bf, isOutput=False)
    out_d = nc.declare_dram_parameter("outT", [4, nwc, 192, N], f32, isOutput=True)

    lam = (lam_sa, lam_ca)

    with TileContext(nc) as tc:
        with (
            tc.tile_pool(name="const", bufs=1) as cpool,
            tc.tile_pool(name="zin", bufs=2) as zpool,
            tc.tile_pool(name="fm", bufs=2) as fmpool,
            tc.tile_pool(name="vt", bufs=2) as vpool,
            tc.tile_pool(name="escore", bufs=2) as epool,
            tc.tile_pool(name="small", bufs=2) as spool,
            tc.tile_pool(name="otile", bufs=2) as opool,
            tc.tile_pool(name="ps_lin", bufs=1, space="PSUM") as ps_lin,
            tc.tile_pool(name="ps_sc", bufs=1, space="PSUM") as ps_sc,
            tc.tile_pool(name="ps_tr", bufs=1, space="PSUM") as ps_tr,
            tc.tile_pool(name="ps_u", bufs=1, space="PSUM") as ps_u,
        ):
            # ---- constants ----
            wfm_t = [[[None] * 3 for _ in range(6)] for _ in range(2)]
            for s in range(2):
                for oc in range(6):
                    for kc in range(3):
                        t = cpool.tile([128, 128], bf, tag=f"wfm{s}{oc}{kc}")
                        nc.sync.dma_start(out=t[:], in_=wfm_d[s, oc, kc])
                        wfm_t[s][oc][kc] = t
            wv_t = []
            for kc in range(3):
                t = cpool.tile([128, 768], bf, tag=f"wv{kc}")
                nc.sync.dma_start(out=t[:], in_=wv_d[kc])
                wv_t.append(t)
            eb_t = cpool.tile([128, 2048], bf, tag="eb")
            nc.sync.dma_start(out=eb_t[:], in_=eb_d[:, :])
            pt_t = []
            for br in range(2):
                pk = []
                for kc in range(2):
                    t = cpool.tile([96, 192], bf, tag=f"pt{br}{kc}")
                    nc.sync.dma_start(out=t[:], in_=pt_d[br, kc * 96:(kc + 1) * 96, :])
                    pk.append(t)
                pt_t.append(pk)
            id_t = cpool.tile([128, 128], bf, tag="ident")
            nc.sync.dma_start(out=id_t[:], in_=id_d[:, :])

            for g in range(n_groups):
                tok0 = g * GROUP * N
                T = GROUP * N  # 512
                zt = []
                for kc in range(3):
                    t = zpool.tile([128, T], bf, tag=f"z{kc}")
                    nc.sync.dma_start(out=t[:], in_=zt_d[kc, :, tok0:tok0 + T])
                    zt.append(t)

                # ---- front-end q/k feature-major ----
                fm = [[None] * 6 for _ in range(2)]
                for s in range(2):
                    for oc in range(6):
                        ps = ps_lin.tile([128, T], f32, tag="lin")
                        for kc in range(3):
                            nc.tensor.matmul(ps[:], wfm_t[s][oc][kc][:], zt[kc][:],
                                             start=(kc == 0), stop=(kc == 2))
                        sb = fmpool.tile([128, T], bf, tag=f"fm{s}{oc}")
                        nc.any.tensor_copy(sb[:], ps[:])
                        fm[s][oc] = sb

                for tb in range(UNITS_PER_GROUP):
                    c0 = tb * 128
                    # ---- v token-major for this unit (2 windows) ----
                    vps = ps_lin.tile([128, 1024], f32, tag="lin")
                    for kc in range(3):
                        nc.tensor.matmul(vps[:, 0:384], zt[kc][:, c0:c0 + 128],
                                         wv_t[kc][:, 0:384],
                                         start=(kc == 0), stop=(kc == 2))
                        nc.tensor.matmul(vps[:, 512:896], zt[kc][:, c0:c0 + 128],
                                         wv_t[kc][:, 384:768],
                                         start=(kc == 0), stop=(kc == 2))
                    # v tiles [128, 6*33] per (br, s) with ones col
                    vt = [[None, None], [None, None]]
                    for br in range(2):
                        for s in range(2):
                            t = vpool.tile([128, 6 * 33], bf, tag=f"v{br}{s}")
                            tv = t[:].rearrange("p (h c) -> p h c", c=33)
                            src = vps[:, s * 512 + br * 192:s * 512 + br * 192 + 192]
                            nc.any.tensor_copy(
                                tv[:, :, 0:32],
                                src.rearrange("p (h c) -> p h c", c=32))
                            nc.vector.memset(tv[:, :, 32:33], 1.0)
                            vt[br][s] = t

                    # ---- scores, BOTH branches in one psum phase ----
                    # col = strip*512 + br*256 + idx2*128 + s*64 (w on partitions)
                    sc = ps_sc.tile([128, 2048], f32, tag="scpr")
                    filled = set()
                    for br in range(2):
                        for h in range(H):
                            if h < 4:
                                qoc, koc = (0, 1) if br == 0 else (4, 5)
                            else:
                                qoc, koc = 2, 3
                            st, i2 = _strip(h, br)
                            off = 32 * st
                            for s in range(2):
                                pc = st * 512 + br * 256 + i2 * 128 + s * 64
                                for w in range(2):
                                    cols = slice(c0 + w * 64, c0 + w * 64 + 64)
                                    q_ap = fm[s][qoc][off:off + 32, cols]
                                    k_ap = fm[s][koc][off:off + 32, cols]
                                    nc.tensor.matmul(
                                        sc[w * 64:w * 64 + 64, pc:pc + 64],
                                        k_ap, q_ap, start=True, stop=True,
                                        tile_position=(off, w * 64))
                                    filled.add(pc)
                    for st in range(4):
                        off = 32 * st
                        for slot in range(8):
                            pc = st * 512 + slot * 64
                            if pc in filled:
                                continue
                            for w in range(2):
                                cols = slice(c0 + w * 64, c0 + w * 64 + 64)
                                d_ap = fm[0][0][off:off + 32, cols]
                                nc.tensor.matmul(
                                    sc[w * 64:w * 64 + 64, pc:pc + 64],
                                    d_ap, d_ap, start=True, stop=True,
                                    tile_position=(off, w * 64))
                    # ---- exp + bias, one pass for both branches ----
                    ex = epool.tile([128, 2048], bf, tag="ex")
                    nc.scalar.activation(ex[:], sc[:], Exp)
                    ebx = epool.tile([128, 2048], bf, tag="ebx")
                    nc.vector.tensor_mul(ebx[:], ex[:], eb_t[:])

                    # ---- AV both branches: pr bank pair per (br, h-half) ----
                    pr = ps_sc.tile([128, 2048], f32, tag="scpr")
                    for br in range(2):
                        for h in range(H):
                            st, i2 = _strip(h, br)
                            pcq = st * 512 + br * 256 + i2 * 128
                            base = ((h % 3) * 132 + (512 if h >= 3 else 0)
                                    + 1024 * br)
                            for w in range(2):
                                et = ebx[w * 64:w * 64 + 64, pcq:pcq + 64]
                                er = ebx[w * 64:w * 64 + 64, pcq + 64:pcq + 128]
                                rows = slice(w * 64, w * 64 + 64)
                                vt_sl = vt[br][0][rows].rearrange(
                                    "p (h c) -> p h c", c=33)[:, h, :]
                                vr_sl = vt[br][1][rows].rearrange(
                                    "p (h c) -> p h c", c=33)[:, h, :]
                                o = w * 64
                                tp = (w * 64, w * 64)
                                for j, (ee, vv) in enumerate(
                                        [(et, vt_sl), (er, vt_sl),
                                         (er, vr_sl), (et, vr_sl)]):
                                    nc.tensor.matmul(
                                        pr[o:o + 64,
                                           base + 33 * j:base + 33 * j + 33],
                                        ee, vv, start=True, stop=True,
                                        tile_position=tp)
                    for br in range(2):
                        # ---- recips (R at col 32 of each 33-block) ----
                        pb = 1024 * br
                        prv0 = pr[:, pb:pb + 396].rearrange(
                            "p (b c) -> p b c", c=33)
                        prv1 = pr[:, pb + 512:pb + 908].rearrange(
                            "p (b c) -> p b c", c=33)
                        rec = spool.tile([128, 24], f32, tag="rec")
                        nc.vector.reciprocal(rec[:, 0:12], prv0[:, :, 32])
                        nc.vector.reciprocal(rec[:, 12:24], prv1[:, :, 32])
                        recl = spool.tile([128, 24], f32, tag="recl")
                        nc.vector.tensor_scalar_mul(recl[:], rec[:], float(lam[br]))

                        # ---- normalize + combine ----
                        # per half: blocks A,D,B,C per head (3 heads/half)
                        tA = opool.tile([128, 384], f32, tag="tA")
                        tD = opool.tile([128, 384], f32, tag="tD")
                        av = tA[:].rearrange("p (h c) -> p h c", c=32)
                        dv = tD[:].rearrange("p (h c) -> p h c", c=32)
                        for half, prv in enumerate([prv0, prv1]):
                            pa = prv[:, :, 0:32].rearrange(
                                "p (h f) c -> p h f c", f=4)
                            rc4 = rec[:, 12 * half:12 * half + 12].rearrange(
                                "p (h f) -> p h f", f=4)
                            rl4 = recl[:, 12 * half:12 * half + 12].rearrange(
                                "p (h f) -> p h f", f=4)
                            ha = 3 * half
                            nc.vector.tensor_mul(
                                av[:, ha:ha + 3, :], pa[:, :, 0, :],
                                rc4[:, :, 0:1].broadcast_to([128, 3, 32]))
                            nc.vector.tensor_mul(
                                dv[:, ha:ha + 3, :], pa[:, :, 1, :],
                                rl4[:, :, 1:2].broadcast_to([128, 3, 32]))
                            nc.vector.tensor_mul(
                                av[:, 6 + ha:6 + ha + 3, :], pa[:, :, 2, :],
                                rc4[:, :, 2:3].broadcast_to([128, 3, 32]))
                            nc.vector.tensor_mul(
                                dv[:, 6 + ha:6 + ha + 3, :], pa[:, :, 3, :],
                                rl4[:, :, 3:4].broadcast_to([128, 3, 32]))
                        oc_t = opool.tile([128, 384], bf, tag="oc")
                        nc.vector.tensor_sub(oc_t[:], tA[:], tD[:])

                        # ---- transpose to feature-major ----
                        trp = ps_tr.tile([128, 512], bf, tag="trp")
                        for ch in range(4):
                            nc.tensor.transpose(
                                trp[0:96, ch * 128:(ch + 1) * 128],
                                oc_t[:, ch * 96:(ch + 1) * 96], id_t[:])
                        otT = opool.tile([96, 512], bf, tag="otT")
                        nc.any.tensor_copy(otT[:], trp[0:96, :])

                        # ---- proj + output ----
                        for st in range(2):
                            ups = ps_u.tile([128, 256], f32, tag="u")
                            for ocn in range(2):
                                for kc in range(2):
                                    mv = otT[:, st * 256 + kc * 128:
                                             st * 256 + (kc + 1) * 128]
                                    wk = pt_t[br][kc]
                                    if ocn == 0:
                                        nc.tensor.matmul(
                                            ups[:, 0:128], wk[:, 0:128], mv,
                                            start=(kc == 0), stop=(kc == 1))
                                    else:
                                        nc.tensor.matmul(
                                            ups[0:64, 128:256], wk[:, 128:192], mv,
                                            start=(kc == 0), stop=(kc == 1))
                            ou = opool.tile([128, 256], f32, tag="ou")
                            nc.any.tensor_copy(ou[:, 0:128], ups[:, 0:128])
                            nc.any.tensor_copy(ou[0:64, 128:256],
                                               ups[0:64, 128:256])
                            qd = br * 2 + st
                            w1 = g * GROUP + tb * 2
                            for w in range(2):
                                nc.sync.dma_start(
                                    out=out_d[qd, w1 + w, 0:128, :],
                                    in_=ou[:, w * 64:w * 64 + 64])
                                nc.sync.dma_start(
                                    out=out_d[qd, w1 + w, 128:192, :],
                                    in_=ou[0:64, 128 + w * 64:128 + w * 64 + 64])
    nc.finalize()
    return nc


def _get_compiled(lam_sa, lam_ca):
    key = (round(lam_sa, 9), round(lam_ca, 9))
    if key not in _COMPILED:
        _COMPILED[key] = _build_nc(lam_sa, lam_ca)
    return _COMPILED[key]


def _run(nc, in_maps):
    from concourse.bass_utils import run_bass_kernel_spmd
    res = run_bass_kernel_spmd(nc, in_maps, list(range(N_CORES)))
    return res.results


def kernel(**inputs):
    shared, zs, (lam_sa, lam_ca) = _host_prep(inputs)
    nc = _get_compiled(lam_sa, lam_ca)
    in_maps = [{"zt": zs[c], **shared} for c in range(N_CORES)]
    results = _run(nc, in_maps)
    out = np.empty((4 * NW, N, DIM), dtype=F32)
    for c in range(N_CORES):
        o = results[c]["outT"]
        w0 = c * NWC
        # quarters: 0=sa_t, 1=sa_r; ca_out is interleaved (2b -> ca_t, 2b+1 -> ca_r)
        out[w0:w0 + NWC] = o[0].transpose(0, 2, 1)
        out[NW + w0:NW + w0 + NWC] = o[1].transpose(0, 2, 1)
        out[2 * NW + 2 * w0:2 * NW + 2 * (w0 + NWC):2] = o[2].transpose(0, 2, 1)
        out[2 * NW + 2 * w0 + 1:2 * NW + 2 * (w0 + NWC):2] = o[3].transpose(0, 2, 1)
    return out



# revision 12
# speedup vs baseline: 90.1523x; 90.1523x over previous
"""Trainium2 Bass kernel for DifferentialDualAttentionInteractiveBlock.

Self-contained: hardcodes shapes (nW=1024, N=64, DIM=192, H=6, d=32),
shards data-parallel over windows across 8 NeuronCores.

v3 layout:
  - scores: per-strip PSUM tiles [128,384], 12 real matmuls per strip,
    no filler; exp+bias per strip (pipelines Act behind PE).
  - AV: per (br,h,w,side) one matmul vs v-concat [vt|1|vr|1] (66 wide):
    48 matmuls/unit instead of 96; per-branch PSUM accumulators.
  - output: per-group SBUF staging (bf16), 2 large DMAs per group
    (32 output DMAs/core instead of 1024).
  - software pipelining: combine/transpose/proj of unit u emitted after
    scores/AV of unit u+1 so PE never waits on the DVE/Act tail.
"""
import sys

sys.path.insert(0, "/opt/trn_rl_repo")

import numpy as np
import ml_dtypes

import concourse.bass as bass  # noqa: F401
import concourse.bacc as bacc
import concourse.mybir as mybir
from concourse.tile import TileContext

BF16 = ml_dtypes.bfloat16
F32 = np.float32

WS = 8
N = 64
DIM = 192
H = 6
HD = 32
SCALE = HD ** -0.5
N_CORES = 8
NW = 1024
NWC = NW // N_CORES
GROUP = 8
UNITS_PER_GROUP = GROUP // 2
N_GROUPS = NWC // GROUP
TOK_C = NWC * N

_COMPILED = {}

# strip st hosts slot-pairs j=0,1,2 -> (br, h)
STRIP_MAP = [
    [(0, 0), (1, 0), (0, 4)],
    [(0, 1), (1, 1), (0, 5)],
    [(0, 2), (1, 2), (1, 4)],
    [(0, 3), (1, 3), (1, 5)],
]
SLOT_OF = {}
for _st, _slots in enumerate(STRIP_MAP):
    for _j, _bh in enumerate(_slots):
        SLOT_OF[_bh] = (_st, _j)


def _qk_ocs(br, h):
    if h < 4:
        return (0, 1) if br == 0 else (4, 5)
    return 2, 3


def _rel_pos_bias(rpb_table):
    coords = np.stack(np.meshgrid(np.arange(WS), np.arange(WS), indexing="ij"))
    cf = coords.reshape(2, -1)
    rc = (cf[:, :, None] - cf[:, None, :]).transpose(1, 2, 0).astype(np.int64)
    rc[:, :, 0] += WS - 1
    rc[:, :, 1] += WS - 1
    rc[:, :, 0] *= 2 * WS - 1
    idx = rc.sum(-1)
    bias = np.asarray(rpb_table)[idx.reshape(-1)].reshape(N, N, H)
    return bias.transpose(2, 0, 1).astype(np.float64)  # [H, q, k]


def _sigmoid_clip(lam):
    s = 1.0 / (1.0 + np.exp(-np.float64(lam)))
    return float(np.clip(s, 0.01, 0.99))


def _host_prep(inputs):
    x = np.asarray(inputs["x_windows"], dtype=F32)
    y = np.asarray(inputs["y_windows"], dtype=F32)

    qkv = np.asarray(inputs["sa_qkv_w"], dtype=np.float64)
    sa_ct = np.asarray(inputs["sa_ct_w"], dtype=np.float64)
    sa_cr = np.asarray(inputs["sa_cr_w"], dtype=np.float64)
    ca_q = np.asarray(inputs["ca_q_w"], dtype=np.float64)
    ca_kv = np.asarray(inputs["ca_kv_w"], dtype=np.float64)
    ca_ct = np.asarray(inputs["ca_ct_w"], dtype=np.float64)
    ca_cr = np.asarray(inputs["ca_cr_w"], dtype=np.float64)
    sa_enh = float(np.asarray(inputs["sa_enh"]))
    ca_enh = float(np.asarray(inputs["ca_enh"]))

    Wq_sa = qkv[0:DIM] * SCALE
    Wk_sa = qkv[DIM:2 * DIM]
    Wv_sa = qkv[2 * DIM:3 * DIM]
    Wq_ca = ca_q * SCALE
    Wk_ca = ca_kv[0:DIM]
    Wv_ca = ca_kv[DIM:2 * DIM]

    def ct(W, enh, cross):
        return np.concatenate([W, enh * (W @ cross)], axis=1)

    def cr(W, enh, cross):
        return np.concatenate([enh * (W @ cross), W], axis=1)

    W_fm_t = np.concatenate([
        ct(Wq_sa, sa_enh, sa_cr), ct(Wk_sa, sa_enh, sa_cr),
        ct(Wq_ca, ca_enh, ca_cr), ct(Wk_ca, ca_enh, ca_cr)], axis=0)
    W_fm_r = np.concatenate([
        cr(Wq_sa, sa_enh, sa_ct), cr(Wk_sa, sa_enh, sa_ct),
        cr(Wq_ca, ca_enh, ca_ct), cr(Wk_ca, ca_enh, ca_ct)], axis=0)
    # oc0=q_sa h0-3, oc1=k_sa h0-3, oc2=[q4s,q5s,q4c,q5c], oc3=[k4s,k5s,k4c,k5c],
    # oc4=q_ca h0-3, oc5=k_ca h0-3
    perm = np.concatenate([
        np.arange(0, 128), np.arange(192, 320),
        np.arange(128, 192), np.arange(384 + 128, 384 + 192),
        np.arange(320, 384), np.arange(576 + 128, 576 + 192),
        np.arange(384, 512), np.arange(576, 704)])
    W_fm_t = W_fm_t[perm]
    W_fm_r = W_fm_r[perm]
    wfm = np.zeros((2, 6, 3, 128, 128), dtype=BF16)
    for s, W in enumerate([W_fm_t, W_fm_r]):
        for oc in range(6):
            for kc in range(3):
                blk = W[oc * 128:(oc + 1) * 128, kc * 128:(kc + 1) * 128]
                wfm[s, oc, kc] = blk.T.astype(BF16)

    W_v_t = np.concatenate([ct(Wv_sa, sa_enh, sa_cr),
                            ct(Wv_ca, ca_enh, ca_cr)], axis=0)
    W_v_r = np.concatenate([cr(Wv_sa, sa_enh, sa_ct),
                            cr(Wv_ca, ca_enh, ca_ct)], axis=0)
    # wv[kc] : [128, 768] = [t-side (384) | r-side (384)], per side sa|ca
    wv = np.zeros((3, 128, 768), dtype=BF16)
    for kc in range(3):
        wv[kc, :, 0:384] = W_v_t[:, kc * 128:(kc + 1) * 128].T.astype(BF16)
        wv[kc, :, 384:768] = W_v_r[:, kc * 128:(kc + 1) * 128].T.astype(BF16)

    bias = _rel_pos_bias(inputs["rpb_table"])
    # log bias (added to scores in PSUM): col = 384*st + 128*j + 64*s + q
    eb = np.zeros((128, 1536), dtype=BF16)
    for st in range(4):
        for j, (br, h) in enumerate(STRIP_MAP[st]):
            e = bias[h].T.astype(BF16)  # [k, q]
            for s in range(2):
                c = 384 * st + 128 * j + 64 * s
                eb[0:64, c:c + 64] = e
                eb[64:128, c:c + 64] = e

    pt = np.zeros((2, 192, 192), dtype=BF16)
    pt[0] = np.asarray(inputs["proj_sa_w"], dtype=np.float64).T.astype(BF16)
    pt[1] = np.asarray(inputs["proj_ca_w"], dtype=np.float64).T.astype(BF16)

    ident = np.eye(128, dtype=BF16)

    lam_sa = _sigmoid_clip(inputs["lambda_sa"])
    lam_ca = _sigmoid_clip(inputs["lambda_ca"])

    # ---- pack weights row-aligned into [146, 8192] ----
    wrows = np.zeros((146, 8192), dtype=BF16)
    flat = wrows.reshape(-1)
    flat[0:589824] = wfm.reshape(-1)                     # rows 0:72
    flat[589824:589824 + 294912] = wv.reshape(-1)        # rows 72:108
    flat[884736:884736 + 196608] = eb.reshape(-1)        # rows 108:132
    for i in range(4):                                   # rows 132:144, 3 each
        br, fh = divmod(i, 2)
        blk = np.zeros((96, 256), dtype=BF16)
        blk[:, 0:192] = pt[br, fh * 96:(fh + 1) * 96, :]
        flat[(132 + 3 * i) * 8192:(135 + 3 * i) * 8192] = blk.reshape(-1)
    flat[144 * 8192:144 * 8192 + 16384] = ident.reshape(-1)  # rows 144:146

    zs = []
    for c in range(N_CORES):
        z = np.concatenate([x[c * NWC:(c + 1) * NWC], y[c * NWC:(c + 1) * NWC]],
                           axis=-1)
        zT = z.reshape(NWC * N, 384).T.astype(BF16).reshape(384, TOK_C)
        zin = np.concatenate([zT, wrows], axis=0)  # [530, 8192]
        zs.append(np.ascontiguousarray(zin))

    shared = {"wfm": wfm, "wv": wv, "eb": eb, "pt": pt, "ident": ident}
    return shared, zs, (lam_sa, lam_ca)


def _build_nc(lam_sa, lam_ca, nwc=NWC):
    n_groups = nwc // GROUP
    n_units = n_groups * UNITS_PER_GROUP
    tok_c = nwc * N
    nc = bacc.Bacc(None, target_bir_lowering=False)
    bf = mybir.dt.bfloat16
    f32 = mybir.dt.float32
    Exp = mybir.ActivationFunctionType.Exp

    zin_d = nc.declare_dram_parameter("zin", [384 + 146, 8192], bf,
                                      isOutput=False)
    # out: feats 0:192 on dim 1.  cols = u(4) x br(2) x side(2) x w(2) x q(64)
    out_d = nc.declare_dram_parameter("out", [n_groups, 192, 2048], bf,
                                      isOutput=True)

    lam = (lam_sa, lam_ca)

    with TileContext(nc) as tc:
        with (
            tc.tile_pool(name="const", bufs=1) as cpool,
            tc.tile_pool(name="zin", bufs=2) as zpool,
            tc.tile_pool(name="fm", bufs=2) as fmpool,
            tc.tile_pool(name="vt", bufs=2) as vpool,
            tc.tile_pool(name="ebx", bufs=8) as ebxpool,
            tc.tile_pool(name="small", bufs=4) as spool,
            tc.tile_pool(name="otile", bufs=2) as opool,
            tc.tile_pool(name="ot2", bufs=4) as opool2,
            tc.tile_pool(name="og", bufs=2) as ogpool,
            tc.tile_pool(name="ps_a", bufs=2, space="PSUM") as ps_a,
            tc.tile_pool(name="ps_u", bufs=2, space="PSUM") as ps_u,
            tc.tile_pool(name="ps_pr", bufs=1, space="PSUM") as ps_pr,
        ):
            # ---- constants ----
            W0 = 384
            wfm_t = [[[None] * 3 for _ in range(6)] for _ in range(2)]
            for s in range(2):
                for oc in range(6):
                    for kc in range(3):
                        t = cpool.tile([128, 128], bf, tag=f"wfm{s}{oc}{kc}")
                        r = W0 + ((s * 6 + oc) * 3 + kc) * 2
                        nc.sync.dma_start(out=t[:], in_=zin_d[r:r + 2, :])
                        wfm_t[s][oc][kc] = t
            wv_t = []
            for kc in range(3):
                t = cpool.tile([128, 768], bf, tag=f"wv{kc}")
                nc.sync.dma_start(out=t[:], in_=zin_d[W0 + 72 + kc * 12:
                                                     W0 + 72 + kc * 12 + 12, :])
                wv_t.append(t)
            eb_t = cpool.tile([128, 1536], bf, tag="eb")
            nc.sync.dma_start(out=eb_t[:], in_=zin_d[W0 + 108:W0 + 132, :])
            pt_t = []
            for br in range(2):
                pk = []
                for fh in range(2):
                    t = cpool.tile([96, 256], bf, tag=f"pt{br}{fh}")
                    r = W0 + 132 + (br * 2 + fh) * 3
                    nc.sync.dma_start(out=t[:], in_=zin_d[r:r + 3, :])
                    pk.append(t)
                pt_t.append(pk)
            id_t = cpool.tile([128, 128], bf, tag="ident")
            nc.sync.dma_start(out=id_t[:], in_=zin_d[W0 + 144:W0 + 146, :])
            # lamv: (b,k,z) interleaved: idx=(3b+k) -> br=idx//6; z in {t,r}
            lamv = cpool.tile([128, 24], f32, tag="lamv")
            nc.vector.memset(lamv[:, 0:12], float(lam[0]))
            nc.vector.memset(lamv[:, 12:24], float(lam[1]))

            # ---- per-group zt prefetch ----
            zts = [None] * n_groups

            def load_zt(g):
                t3 = []
                for kc in range(3):
                    t = zpool.tile([128, GROUP * N], bf, tag=f"z{kc}")
                    nc.sync.dma_start(
                        out=t[:],
                        in_=zin_d[kc * 128:(kc + 1) * 128,
                                  g * GROUP * N:(g + 1) * GROUP * N])
                    t3.append(t)
                zts[g] = t3

            fms = [None] * n_groups

            def emit_fm(g):
                fm = [[None] * 6 for _ in range(2)]
                for s in range(2):
                    for oc in range(6):
                        ps = ps_a.tile([128, GROUP * N], f32, tag="pa")
                        for kc in range(3):
                            nc.tensor.matmul(ps[:], wfm_t[s][oc][kc][:],
                                             zts[g][kc][:],
                                             start=(kc == 0), stop=(kc == 2))
                        sb = fmpool.tile([128, GROUP * N], bf, tag=f"fm{s}{oc}")
                        nc.any.tensor_copy(sb[:], ps[:])
                        fm[s][oc] = sb
                fms[g] = fm

            ogs = [None] * n_groups
            state = [None] * n_units

            def stage_A(u):
                """v, scores (+bias prefill), exp for unit u."""
                g, tb = divmod(u, UNITS_PER_GROUP)
                c0 = tb * 128
                fm = fms[g]
                zt = zts[g]
                st_d = {}
                # ---- v token-major, per-side psum, per-br concat tiles ----
                vps = []
                for s in range(2):
                    ps = ps_a.tile([128, 384], f32, tag="pa")
                    for kc in range(3):
                        nc.tensor.matmul(ps[:], zt[kc][:, c0:c0 + 128],
                                         wv_t[kc][:, 384 * s:384 * s + 384],
                                         start=(kc == 0), stop=(kc == 2))
                    vps.append(ps)
                vt = []
                for br in range(2):
                    t = vpool.tile([128, 6 * 66], bf, tag=f"v{br}")
                    tv = t[:].rearrange("p (h c) -> p h c", c=66)
                    for s in range(2):
                        src = vps[s][:, br * 192:br * 192 + 192]
                        nc.any.tensor_copy(
                            tv[:, :, s * 33:s * 33 + 32],
                            src.rearrange("p (h c) -> p h c", c=32))
                        nc.gpsimd.memset(tv[:, :, s * 33 + 32:s * 33 + 33], 1.0)
                    vt.append(t)
                st_d["vt"] = vt

                # ---- scores per strip (bias prefilled in PSUM) + exp ----
                ebx = [None] * 4
                for st in range(4):
                    sc = ps_a.tile([128, 384], f32, tag="pa")
                    nc.tensor.matmul(sc[:], id_t[:],
                                     eb_t[:, 384 * st:384 * st + 384],
                                     start=True, stop=True)
                    off = 32 * st
                    for j, (br, h) in enumerate(STRIP_MAP[st]):
                        qoc, koc = _qk_ocs(br, h)
                        for s in range(2):
                            for w in range(2):
                                cols = slice(c0 + w * 64, c0 + w * 64 + 64)
                                q_ap = fm[s][qoc][off:off + 32, cols]
                                k_ap = fm[s][koc][off:off + 32, cols]
                                nc.tensor.matmul(
                                    sc[w * 64:w * 64 + 64,
                                       j * 128 + s * 64:j * 128 + s * 64 + 64],
                                    k_ap, q_ap, start=False, stop=False,
                                    skip_group_check=True,
                                    tile_position=(off, w * 64))
                    bx = ebxpool.tile([128, 384], bf, tag="ebx")
                    nc.scalar.activation(bx[:], sc[:], Exp)
                    ebx[st] = bx
                st_d["ebx"] = ebx
                state[u] = st_d

            def stage_B(u):
                """AV matmuls for unit u: one pr tile [128, 2048]."""
                st_d = state[u]
                vt = st_d["vt"]
                ebx = st_d["ebx"]
                pr = ps_pr.tile([128, 2048], f32, tag="pr")
                for br in range(2):
                    for h in range(H):
                        st, j = SLOT_OF[(br, h)]
                        idx = 6 * br + h
                        blk = 512 * (idx // 3) + 132 * (idx % 3)
                        for w in range(2):
                            rows = slice(w * 64, w * 64 + 64)
                            vv = vt[br][rows].rearrange(
                                "p (h c) -> p h c", c=66)[:, h, :]
                            for es in range(2):
                                ee = ebx[st][rows,
                                             j * 128 + es * 64:
                                             j * 128 + es * 64 + 64]
                                nc.tensor.matmul(
                                    pr[rows, blk + es * 66:blk + es * 66 + 66],
                                    ee, vv, start=True, stop=True,
                                    tile_position=(w * 64, w * 64))
                st_d["pr"] = pr

            def stage_CD(u):
                """combine, transpose, proj, output staging for unit u."""
                g, tb = divmod(u, UNITS_PER_GROUP)
                st_d = state[u]
                pr = st_d["pr"]

                def prv4(coff, width=32):
                    ap = pr[:].rearrange("p (b q) -> p b q", b=4)
                    ap = ap[:, :, 0:396].rearrange(
                        "p b (k q) -> p b k q", k=3)
                    return ap[:, :, :, coff:coff + width]

                rec = spool.tile([128, 24], f32, tag="rec")
                rec4 = rec[:].rearrange("p (b k z) -> p b k z", k=3, z=2)
                nc.vector.reciprocal(rec4[:, :, :, 0:1], prv4(32, 1))
                nc.vector.reciprocal(rec4[:, :, :, 1:2], prv4(98, 1))
                lm = spool.tile([128, 24], f32, tag="lm")
                nc.any.tensor_mul(lm[:], rec[:], lamv[:])
                lm4 = lm[:].rearrange("p (b k z) -> p b k z", k=3, z=2)

                bc = lambda a: a.broadcast_to([128, 4, 3, 32])
                tAt = opool.tile([128, 384], f32, tag="tAt")
                tDt = opool.tile([128, 384], f32, tag="tDt")
                tAr = opool.tile([128, 384], f32, tag="tAr")
                tDr = opool.tile([128, 384], f32, tag="tDr")
                v4 = lambda t: t[:].rearrange("p (b k c) -> p b k c", b=4, k=3)
                nc.any.tensor_mul(v4(tAt), prv4(0), bc(rec4[:, :, :, 0:1]))
                nc.any.tensor_mul(v4(tDt), prv4(66), bc(lm4[:, :, :, 1:2]))
                nc.any.tensor_mul(v4(tAr), prv4(99), bc(rec4[:, :, :, 1:2]))
                nc.any.tensor_mul(v4(tDr), prv4(33), bc(lm4[:, :, :, 0:1]))
                oc_t = opool.tile([128, 768], bf, tag="oc")
                ocv = oc_t[:].rearrange(
                    "p (br sd h c) -> p br sd h c", br=2, sd=2, c=32)
                tv6 = lambda t: t[:].rearrange(
                    "p (br h c) -> p br h c", br=2, c=32)
                nc.any.tensor_sub(ocv[:, :, 0], tv6(tAt), tv6(tDt))
                nc.any.tensor_sub(ocv[:, :, 1], tv6(tAr), tv6(tDr))

                # ---- transpose + proj + stage output ----
                for br in range(2):
                    trp = ps_a.tile([128, 512], bf, tag="pa")
                    for sd in range(2):
                        for fh in range(2):
                            nc.tensor.transpose(
                                trp[0:96, 256 * sd + 128 * fh:
                                    256 * sd + 128 * fh + 128],
                                oc_t[:, 384 * br + 192 * sd + 96 * fh:
                                     384 * br + 192 * sd + 96 * fh + 96],
                                id_t[:])
                    otT = opool2.tile([96, 512], bf, tag="otT")
                    nc.any.tensor_copy(otT[:], trp[0:96, :])
                    otv = otT[:].rearrange("p (sd q) -> p sd q", sd=2)

                    ups = ps_u.tile([128, 512], f32, tag="pu")
                    upv = ups[:].rearrange("p (sd q) -> p sd q", sd=4)
                    for fh in range(2):
                        nc.tensor.matmul(
                            upv[:, 0:2], pt_t[br][fh][:, 0:128],
                            otv[:, :, 128 * fh:128 * fh + 128],
                            start=(fh == 0), stop=(fh == 1))
                    for fh in range(2):
                        nc.tensor.matmul(
                            upv[0:64, 2:4], pt_t[br][fh][:, 128:192],
                            otv[:, :, 128 * fh:128 * fh + 128],
                            start=(fh == 0), stop=(fh == 1))
                    ogA, ogB = ogs[g]
                    base = (tb * 2 + br) * 256
                    nc.any.tensor_copy(ogA[:, base:base + 256], ups[:, 0:256])
                    nc.any.tensor_copy(ogB[:, base:base + 256],
                                          ups[0:64, 256:512])
                if tb == UNITS_PER_GROUP - 1:
                    ogA, ogB = ogs[g]
                    nc.sync.dma_start(out=out_d[g, 0:128, :], in_=ogA[:])
                    nc.sync.dma_start(out=out_d[g, 128:192, :], in_=ogB[:])

            # ---- pipelined emission ----
            load_zt(0)
            for u in range(n_units):
                g, tb = divmod(u, UNITS_PER_GROUP)
                if tb == 0:
                    if g + 1 < n_groups:
                        load_zt(g + 1)
                    emit_fm(g)
                    ogA = ogpool.tile([128, 2048], bf, tag="ogA")
                    ogB = ogpool.tile([64, 2048], bf, tag="ogB")
                    ogs[g] = (ogA, ogB)
                stage_A(u)
                if u > 0:
                    stage_CD(u - 1)
                stage_B(u)
                if u > 0:
                    state[u - 1] = None
            stage_CD(n_units - 1)
    nc.finalize()
    return nc


def _get_compiled(lam_sa, lam_ca):
    key = (round(lam_sa, 9), round(lam_ca, 9))
    if key not in _COMPILED:
        _COMPILED[key] = _build_nc(lam_sa, lam_ca)
    return _COMPILED[key]


def _run(nc, in_maps):
    from concourse.bass_utils import run_bass_kernel_spmd
    res = run_bass_kernel_spmd(nc, in_maps, list(range(N_CORES)))
    return res.results


def _assemble(results):
    out = np.empty((4 * NW, N, DIM), dtype=F32)
    for c in range(N_CORES):
        arr = np.asarray(results[c]["out"])  # [16, 192, 2048]
        arr = arr.reshape(N_GROUPS, 192, 4, 2, 2, 2, 64)
        # [g, feat, u, br, side, w, q] -> [br, side, g, u, w, q, feat]
        arr = arr.transpose(3, 4, 0, 2, 5, 6, 1).astype(F32)
        arr = np.ascontiguousarray(arr).reshape(2, 2, NWC, N, DIM)
        w0 = c * NWC
        out[w0:w0 + NWC] = arr[0, 0]
        out[NW + w0:NW + w0 + NWC] = arr[0, 1]
        out[2 * NW + 2 * w0:2 * NW + 2 * (w0 + NWC):2] = arr[1, 0]
        out[2 * NW + 2 * w0 + 1:2 * NW + 2 * (w0 + NWC):2] = arr[1, 1]
    return out


def kernel(**inputs):
    shared, zs, (lam_sa, lam_ca) = _host_prep(inputs)
    nc = _get_compiled(lam_sa, lam_ca)
    in_maps = [{"zin": zs[c]} for c in range(N_CORES)]
    results = _run(nc, in_maps)
    return _assemble(results)
